# revision 42
# baseline (speedup 1.0000x reference)
"""NonLocalBlock (spatial self-attention) Trainium2 Bass kernel.

Problem: x [4, 128, 64, 64]; 1x1 convs theta/phi/g -> softmax(theta^T phi) g
-> 1x1 conv out + residual.

Sharding (8 cores): core k -> (batch b = k//2, query-half h = k%2).
Each core holds the full keys/values for its batch (xkv [128, 4096], rolled
host-side so its 2048 queries are columns [0, 2048)).  Weights replicated.

Key structural ideas:

1. Fused value path, rank-127:  G = w_out @ w_g has sigma_128 ~ 1e-9, so
   G ~= C_out @ P_g with P_g = V^T[:127] and C_out = U[:, :127] * S[:127].
   The PV stationary chunks [m=128, 128] hold column 0 = ones and columns
   1..127 = (P_g x)^T, so a single PV matmul accumulates BOTH the attention
   value sum (rows 1..127) and the softmax denominator (row 0).  No
   dedicated denominator matmuls or reductions anywhere.

2. Host-side normalization:  out = C_out(y/den) + x + b == (C_out y)/den
   + x + b, so the device ships the *unnormalized* conv result and the den
   row; the host does conv/den + x + b in numpy.  No reciprocal /
   partition-broadcast round-trip on device.

3. Two-engine exp: ACT computes exp for 10 of every 16 key-chunk pairs;
   DVE computes the other 6 with a Schraudolph bit-trick in ONE
   tensor_scalar op: i16 = round(s * 128*log2(e) + (127*128 - C)), whose
   int16 bit pattern IS bf16(exp(s)) (~3% max element error, common-mode
   across neighbouring scores so softmax normalization cancels most of it;
   end-to-end sim: 5-6e-3 rel err).  This removes ACT as the pipeline
   pacer; the PE's 512-column matmul stream is the bottleneck.

4. p-state care: TRN2's PE ramps 0.65/1.2 -> 2.4 GHz only after ~3us of
   gapless execution.  Dummy matmuls on a zeroed scratch tile ramp the
   clock while the input DMAs stream, the bf16 projections (host-precast
   x and weights, so no on-device cast chain) keep it hot, and QK runs 3
   pair-steps ahead of exp (s_pool bufs=3, PV delayed 3) so ACT/DVE
   jitter never starves the PE.

Per 512-query block (16 key-chunk pairs, software-pipelined):
  S^T pair [128m, 2, 512n] = phi_chunk^T @ theta_blk  (PSUM, bf16)
  P^T = exp(S^T)  (ACT or DVE, PSUM->SBUF bf16; scores O(30) safe in fp32)
  attn_ps [128, 512] += ghatT_chunk^T @ P^T_chunk  (PSUM accum, bf16)
  epilogue of block b (bf16 cast, conv, DMA out) emitted early in block b+1.
"""

import numpy as np

B, C = 4, 128
HW = 4096  # 64*64 spatial positions
QH = HW // 2  # queries per core
NCORES = 8
NBLK = 512  # query block size
NMCH = HW // 128  # 32 key chunks of 128
PVD = 3  # PV trails QK by this many pair-steps (= s_pool bufs)
DVE_PAIRS = {2, 4, 7, 9, 12, 14}  # pair indices handled by the DVE exp

# Schraudolph constants for bf16-via-int16: bitcast_bf16(round_i16(A*s + B))
EXP_A16 = 184.6649652337873  # 2^7 * log2(e)
EXP_B16 = 16250.409332        # 127*128 - 366392.7/65536

_CACHE = {}


def _legalize_waits(bir, verbose=False):
    """Split instructions carrying more sync waits than the gen3 ISA allows.

    Walrus caps sync waits at 1 per instruction (2 for EventSemaphore); the
    Tile tail drain and first-consumer instructions can exceed that. Spill
    excess waits onto inserted wait-only EventSemaphore instructions placed
    immediately before the offender on the same engine (engines execute
    in order, so this is semantics-preserving).
    """
    n_split = 0
    where = []
    for f in bir["functions"]:
        for bb in f["blocks"]:
            out = []
            for inst in bb["instructions"]:
                si = inst.get("sync_info")
                waits = (si or {}).get("on_wait") or []
                cap = 2 if inst["opcode"] == "EventSemaphore" else 1
                if len(waits) > cap:
                    excess = waits[:-cap]
                    si["on_wait"] = waits[-cap:]
                    for i in range(0, len(excess), 2):
                        chunk = excess[i : i + 2]
                        out.append(
                            {
                                "debug": inst.get("debug", 0),
                                "engine": inst["engine"],
                                "ins": [],
                                "name": f'{inst["name"]}_w{i}',
                                "opcode": "EventSemaphore",
                                "outs": [],
                                "sync_info": {"on_update": [], "on_wait": chunk},
                            }
                        )
                        n_split += 1
                    where.append((inst["name"], inst["opcode"], len(excess)))
                out.append(inst)
            bb["instructions"] = out
    if verbose and where:
        print(f"[legalize_waits] {n_split} wait insts inserted for:")
        for nm, op, ne in where:
            print(f"  {nm} ({op}): {ne} excess waits")
    return bir


def _build():
    from contextlib import ExitStack

    import concourse.bass as bass
    import concourse.tile as tile
    from concourse import mybir

    f32 = mybir.dt.float32
    bf16 = mybir.dt.bfloat16
    i16 = mybir.dt.int16

    Exp = mybir.ActivationFunctionType.Exp
    Copy = mybir.ActivationFunctionType.Copy

    nc = bass.Bass()
    # all big inputs pre-cast to bf16 host-side: halves DMA traffic and
    # lets the projections run as bf16 matmuls with no on-device casts
    x_kv = nc.dram_tensor("xkv", [C, HW], bf16, kind="ExternalInput")
    wts_d = nc.dram_tensor("wts", [C, 4 * C], bf16, kind="ExternalInput")
    bias_d = nc.dram_tensor("bias", [C, 2], f32, kind="ExternalInput")
    out_d = nc.dram_tensor("out", [C, QH], f32, kind="ExternalOutput")
    den_d = nc.dram_tensor("den", [QH // NBLK, NBLK], f32, kind="ExternalOutput")

    with ExitStack() as ctx:
        tc = ctx.enter_context(tile.TileContext(nc))
        const = ctx.enter_context(tc.tile_pool(name="const", bufs=1))
        persist = ctx.enter_context(tc.tile_pool(name="persist", bufs=1))
        small = ctx.enter_context(tc.tile_pool(name="small", bufs=2))
        pt_pool = ctx.enter_context(tc.tile_pool(name="pt", bufs=16))

        # ---- loads: the DMA bus serializes the whole input set anyway,
        # so use few big transfers on two parallel issue queues ----
        xkv_s = persist.tile([C, HW], bf16, tag="xkv")
        nc.scalar.dma_start(out=xkv_s[:, 0 : HW // 2], in_=x_kv[:, 0 : HW // 2])
        wts_s = const.tile([C, 4 * C], bf16, tag="wts")
        nc.sync.dma_start(out=wts_s, in_=wts_d[:, :])
        bias_s = const.tile([C, 2], f32, tag="bias")
        nc.sync.dma_start(out=bias_s, in_=bias_d[:, :])
        nc.sync.dma_start(out=xkv_s[:, HW // 2 :], in_=x_kv[:, HW // 2 :])
        xkv_t = [xkv_s[:, j * 512 : (j + 1) * 512] for j in range(8)]
        w_s = {
            nm: wts_s[:, i * C : (i + 1) * C]
            for i, nm in enumerate(("wt", "wp", "pg", "co"))
        }
        b_s = {"bt": bias_s[:, 0:1], "bp": bias_s[:, 1:2]}

        # warm the ACT exp table while DMAs stream (one-time ~1.3us load)
        warm = const.tile([C, 1], f32, tag="warm")
        nc.scalar.activation(out=warm, in_=b_s["bt"], func=Exp, bias=0.0, scale=1.0)

        # warm the PE p-state while DMAs stream: ~4us of dummy matmuls on a
        # zeroed scratch tile ramps the clock 0.65 -> 2.4 GHz before the
        # real projections start (output never read; junk by design)
        scratch = const.tile([C, 512], bf16, tag="scratch")
        nc.gpsimd.memset(scratch, 0.0)

        theta_s = persist.tile([C, QH], bf16, tag="theta")
        phi_t = [
            persist.tile([C, QH], bf16, tag=f"phi{t}", name=f"phi{t}")
            for t in range(2)
        ]
        gn_t = [
            persist.tile([C, QH], bf16, tag=f"gn{t}", name=f"gn{t}")
            for t in range(2)
        ]
        gT_t = [
            persist.tile([128, NMCH // 2, 128], bf16, tag=f"gT{t}", name=f"gT{t}")
            for t in range(2)
        ]

        # ---- projections (bf16 512-col matmuls; PSUM->SBUF drains split
        # between ACT and DVE so neither paces the PE stream) ----
        Ident = mybir.ActivationFunctionType.Identity
        with tc.tile_pool(name="proj_ps", bufs=4, space="PSUM") as proj_ps:
            warm_ps = proj_ps.tile([128, 512], f32, tag="warmps")
            for _ in range(16):
                nc.tensor.matmul(warm_ps, scratch[:, 0:128], scratch,
                                 start=True, stop=True)

            def proj(dst, wsrc, j, bias=None, act=False):
                ps = proj_ps.tile([128, 512], f32, tag="p")
                nc.tensor.matmul(
                    ps, w_s[wsrc], xkv_t[j], start=True, stop=True
                )
                # theta/phi drains all on DVE and ghat drains all on ACT:
                # the first exps (ACT) then only queue behind the 4 ghat
                # drains instead of 10 mixed ones
                if act:
                    nc.scalar.activation(
                        out=dst,
                        in_=ps,
                        func=Ident,
                        bias=b_s[bias] if bias else 0.0,
                        scale=1.0,
                    )
                elif bias is not None:
                    nc.vector.tensor_scalar_add(out=dst, in0=ps, scalar1=b_s[bias])
                else:
                    nc.vector.tensor_copy(out=dst, in_=ps)

            for j in range(4):  # theta over this core's queries
                proj(theta_s[:, j * 512 : (j + 1) * 512], "wt", j, "bt")
            for j in range(8):  # phi over all keys
                proj(
                    phi_t[j // 4][:, (j % 4) * 512 : (j % 4 + 1) * 512],
                    "wp",
                    j,
                    "bp",
                )
            for j in range(8):  # ghat natural layout [k, m]
                proj(
                    gn_t[j // 4][:, (j % 4) * 512 : (j % 4 + 1) * 512],
                    "pg",
                    j,
                    act=True,
                )
                if j % 4 == 3:
                    # transpose this half: [k=128, 2048] -> [m 128, 16, k 128]
                    half = j // 4
                    nc.sync.dma_start_transpose(out=gT_t[half], in_=gn_t[half])
                    # ones channel -> PV row 0 accumulates the denominator
                    nc.vector.memset(gT_t[half][:, :, 0:1], 1.0)

        # ---- attention ----
        s_pool = ctx.enter_context(tc.tile_pool(name="s_ps", bufs=PVD, space="PSUM"))
        attn_pool = ctx.enter_context(tc.tile_pool(name="attn_ps", bufs=1, space="PSUM"))
        conv_pool = ctx.enter_context(tc.tile_pool(name="conv_ps", bufs=1, space="PSUM"))

        pending = None  # (attn_ps, q0, blk) of the previous block

        def finish_block(attn_ps, q0, blk, last=False):
            den_s = small.tile([1, 512], f32, tag="den_s")
            nc.vector.tensor_copy(out=den_s, in_=attn_ps[0:1, :])
            nc.scalar.dma_start(out=den_d[blk : blk + 1, :], in_=den_s)
            if not last:
                yu = small.tile([128, 512], bf16, tag="yu")
                nc.vector.tensor_copy(out=yu, in_=attn_ps)
                conv_ps = conv_pool.tile([128, 512], f32, tag="conv")
                nc.tensor.matmul(conv_ps, w_s["co"], yu, start=True, stop=True)
                out_s = small.tile([128, 512], f32, tag="out_s")
                nc.vector.tensor_copy(out=out_s, in_=conv_ps)
                nc.sync.dma_start(out=out_d[:, q0 : q0 + NBLK], in_=out_s)
            else:
                # tail: half casts run concurrently on ACT and DVE so the
                # first conv starts ~350ns after the last PV
                conv_ps = conv_pool.tile([128, 512], f32, tag="conv")
                yus = []
                for hh in range(2):
                    sl = slice(hh * 256, (hh + 1) * 256)
                    yu = small.tile([128, 256], bf16, tag=f"yu{hh}", name=f"yu{hh}")
                    if hh == 0:
                        nc.scalar.activation(
                            out=yu, in_=attn_ps[:, sl], func=Copy,
                            bias=0.0, scale=1.0,
                        )
                    else:
                        nc.vector.tensor_copy(out=yu, in_=attn_ps[:, sl])
                    yus.append(yu)
                for hh in range(2):
                    sl = slice(hh * 256, (hh + 1) * 256)
                    nc.tensor.matmul(
                        conv_ps[:, sl], w_s["co"], yus[hh], start=True, stop=True
                    )
                    out_s = small.tile(
                        [128, 256], f32, tag=f"out_s{hh}", name=f"out_s{hh}"
                    )
                    nc.vector.tensor_copy(out=out_s, in_=conv_ps[:, sl])
                    nc.sync.dma_start(
                        out=out_d[:, q0 + hh * 256 : q0 + (hh + 1) * 256],
                        in_=out_s,
                    )

        NPAIR = NMCH // 2
        for blk in range(QH // NBLK):
            q0 = blk * NBLK
            thq = theta_s[:, q0 : q0 + NBLK]
            pt_tiles = []
            attn_ps = attn_pool.tile([128, 512], f32, tag="attn")
            # QK/exp of pair pj runs PVD steps ahead of PV of pair pj-PVD.
            for pj in range(NPAIR + PVD):
                if pj < NPAIR:
                    sp = s_pool.tile([128, 2, 512], f32, tag="s")
                    for k2 in range(2):
                        mi = pj * 2 + k2
                        nc.tensor.matmul(
                            sp[:, k2, :],
                            phi_t[mi // 16][:, (mi % 16) * 128 : (mi % 16 + 1) * 128],
                            thq,
                            start=True,
                            stop=True,
                        )
                    pt = pt_pool.tile([128, 2, 512], bf16, tag="pt")
                    if pj in DVE_PAIRS:
                        # Schraudolph exp on DVE: int16(A*s+B) bits == bf16 P
                        nc.vector.tensor_scalar(
                            out=pt.bitcast(i16),
                            in0=sp,
                            scalar1=EXP_A16,
                            scalar2=EXP_B16,
                            op0=mybir.AluOpType.mult,
                            op1=mybir.AluOpType.add,
                        )
                    else:
                        nc.scalar.activation(
                            out=pt, in_=sp, func=Exp, bias=0.0, scale=1.0
                        )
                    pt_tiles.append(pt)
                if pj == 2 and pending is not None:
                    finish_block(*pending)
                if pj >= PVD:
                    p = pj - PVD
                    for k2 in range(2):
                        mi = p * 2 + k2
                        nc.tensor.matmul(
                            attn_ps,
                            gT_t[mi // 16][:, mi % 16, :],
                            pt_tiles[p][:, k2, :],
                            start=(mi == 0),
                            stop=(mi == NMCH - 1),
                        )
            pending = (attn_ps, q0, blk)
        finish_block(*pending, last=True)

    # populate .instr bytes for extended-inst InstISA subclasses — raw Bass
    # skips this pass and the NEFF compiler fails "ISA wrong length"
    mybir.codegen_inst_isa_subclasses(nc)

    import json as _json
    import os as _os

    blob = _json.dumps(
        _legalize_waits(
            _json.loads(nc.to_json_bytes()),
            verbose=bool(_os.environ.get("KERNEL_DEBUG")),
        )
    ).encode()
    nc.to_json_bytes = lambda: blob
    return nc


def _get_nc():
    if "nc" not in _CACHE:
        _CACHE["nc"] = _build()
    return _CACHE["nc"]


def _prep_host(inputs):
    """Host-side precompute: weight transposes, fused G = w_out@w_g SVD
    split (rank 127 + ones/denominator channel at k=0), fused bias, and
    bf16 casts + packing of all device weight inputs."""
    import ml_dtypes

    bf16 = ml_dtypes.bfloat16
    w_g = np.asarray(inputs["w_g"], np.float32)
    w_out = np.asarray(inputs["w_out"], np.float32)
    G = w_out @ w_g
    U, S, Vt = np.linalg.svd(G)
    r = 127
    pg = np.zeros((C, C), np.float32)  # lhsT: pg[c, k] = P_g[k-1, c]
    pg[:, 1 : r + 1] = Vt[:r, :].T
    co = np.zeros((C, C), np.float32)  # lhsT: co[k, c] = C_out[c, k-1]
    co[1 : r + 1, :] = (U[:, :r] * S[:r][None, :]).T
    bcomb = (
        np.asarray(inputs["b_out"], np.float32)
        + w_out @ np.asarray(inputs["b_g"], np.float32)
    ).reshape(C, 1)
    wts = np.concatenate(
        [
            np.asarray(inputs["w_theta"], np.float32).T,
            np.asarray(inputs["w_phi"], np.float32).T,
            pg,
            co,
        ],
        axis=1,
    ).astype(bf16)
    bias = np.concatenate(
        [
            np.asarray(inputs["b_theta"], np.float32).reshape(C, 1),
            np.asarray(inputs["b_phi"], np.float32).reshape(C, 1),
        ],
        axis=1,
    )
    wmaps = {
        "wts": np.ascontiguousarray(wts),
        "bias": np.ascontiguousarray(bias),
    }
    return wmaps, bcomb


def _run(inputs, trace=False, **spmd_kwargs):
    import ml_dtypes

    from concourse.bass_utils import run_bass_kernel_spmd

    x = np.asarray(inputs["x"], np.float32)
    xf = np.ascontiguousarray(x.reshape(B, C, HW))
    wmaps, bcomb = _prep_host(inputs)
    in_maps = []
    for k in range(NCORES):
        b, h = k // 2, k % 2
        # rotate keys so this core's queries are columns [0, QH)
        xkv = np.ascontiguousarray(
            np.roll(xf[b], -h * QH, axis=1).astype(ml_dtypes.bfloat16)
        )
        in_maps.append({"xkv": xkv, **wmaps})
    nc = _get_nc()
    res = run_bass_kernel_spmd(
        nc, in_maps, core_ids=list(range(NCORES)), trace=trace, **spmd_kwargs
    )
    out = np.empty((B, C, HW), np.float32)
    for k in range(NCORES):
        b, h = k // 2, k % 2
        conv_u = res.results[k]["out"]  # [C, QH], unnormalized conv result
        den = res.results[k]["den"].reshape(QH)  # softmax denominators
        xq = xf[b][:, h * QH : (h + 1) * QH]
        out[b][:, h * QH : (h + 1) * QH] = conv_u / den[None, :] + xq + bcomb
    return out.reshape(B, C, 64, 64), res


def kernel(**inputs):
    out, _ = _run(inputs, trace=False)
    return out


# revision 43
# speedup vs baseline: 1.0562x; 1.0562x over previous
"""NonLocalBlock (spatial self-attention) Trainium2 Bass kernel.

Problem: x [4, 128, 64, 64]; 1x1 convs theta/phi/g -> softmax(theta^T phi) g
-> 1x1 conv out + residual.

Sharding (8 cores): core k -> (batch b = k//2, query-half h = k%2).
Each core holds the full keys/values for its batch (xkv [128, 4096], rolled
host-side so its 2048 queries are columns [0, 2048)).  Weights replicated.

Key structural ideas:

1. Fused value path, rank-127:  G = w_out @ w_g has sigma_128 ~ 1e-9, so
   G ~= C_out @ P_g with P_g = V^T[:127] and C_out = U[:, :127] * S[:127].
   The PV stationary chunks [m=128, 128] hold column 0 = ones and columns
   1..127 = (P_g x)^T, so a single PV matmul accumulates BOTH the attention
   value sum (rows 1..127) and the softmax denominator (row 0).  No
   dedicated denominator matmuls or reductions anywhere.

2. Host-side normalization:  out = C_out(y/den) + x + b == (C_out y)/den
   + x + b, so the device ships the *unnormalized* conv result and the den
   row; the host does conv/den + x + b in numpy.  No reciprocal /
   partition-broadcast round-trip on device.

3. Two-engine exp: ACT computes exp for 10 of every 16 key-chunk pairs;
   DVE computes the other 6 with a Schraudolph bit-trick in ONE
   tensor_scalar op: i16 = round(s * 128*log2(e) + (127*128 - C)), whose
   int16 bit pattern IS bf16(exp(s)) (~3% max element error, common-mode
   across neighbouring scores so softmax normalization cancels most of it;
   end-to-end sim: 5-6e-3 rel err).  This removes ACT as the pipeline
   pacer; the PE's 512-column matmul stream is the bottleneck.

4. p-state care: TRN2's PE ramps 0.65/1.2 -> 2.4 GHz only after ~3us of
   gapless execution.  Dummy matmuls on a zeroed scratch tile ramp the
   clock while the input DMAs stream, the bf16 projections (host-precast
   x and weights, so no on-device cast chain) keep it hot, and QK runs 3
   pair-steps ahead of exp (s_pool bufs=3, PV delayed 3) so ACT/DVE
   jitter never starves the PE.

Per 512-query block (16 key-chunk pairs, software-pipelined):
  S^T pair [128m, 2, 512n] = phi_chunk^T @ theta_blk  (PSUM, bf16)
  P^T = exp(S^T)  (ACT or DVE, PSUM->SBUF bf16; scores O(30) safe in fp32)
  attn_ps [128, 512] += ghatT_chunk^T @ P^T_chunk  (PSUM accum, bf16)
  epilogue of block b (bf16 cast, conv, DMA out) emitted early in block b+1.
"""

import numpy as np

B, C = 4, 128
HW = 4096  # 64*64 spatial positions
QH = HW // 2  # queries per core
NCORES = 8
NBLK = 512  # query block size
NMCH = HW // 128  # 32 key chunks of 128
PVD = 3  # PV trails QK by this many pair-steps (= s_pool bufs)
DVE_PAIRS = {2, 4, 7, 9, 12, 14}  # pair indices handled by the DVE exp

# Schraudolph constants for bf16-via-int16: bitcast_bf16(round_i16(A*s + B))
EXP_A16 = 184.6649652337873  # 2^7 * log2(e)
EXP_B16 = 16250.409332        # 127*128 - 366392.7/65536

_CACHE = {}


def _legalize_waits(bir, verbose=False):
    """Split instructions carrying more sync waits than the gen3 ISA allows.

    Walrus caps sync waits at 1 per instruction (2 for EventSemaphore); the
    Tile tail drain and first-consumer instructions can exceed that. Spill
    excess waits onto inserted wait-only EventSemaphore instructions placed
    immediately before the offender on the same engine (engines execute
    in order, so this is semantics-preserving).
    """
    n_split = 0
    where = []
    for f in bir["functions"]:
        for bb in f["blocks"]:
            out = []
            for inst in bb["instructions"]:
                si = inst.get("sync_info")
                waits = (si or {}).get("on_wait") or []
                cap = 2 if inst["opcode"] == "EventSemaphore" else 1
                if len(waits) > cap:
                    excess = waits[:-cap]
                    si["on_wait"] = waits[-cap:]
                    for i in range(0, len(excess), 2):
                        chunk = excess[i : i + 2]
                        out.append(
                            {
                                "debug": inst.get("debug", 0),
                                "engine": inst["engine"],
                                "ins": [],
                                "name": f'{inst["name"]}_w{i}',
                                "opcode": "EventSemaphore",
                                "outs": [],
                                "sync_info": {"on_update": [], "on_wait": chunk},
                            }
                        )
                        n_split += 1
                    where.append((inst["name"], inst["opcode"], len(excess)))
                out.append(inst)
            bb["instructions"] = out
    if verbose and where:
        print(f"[legalize_waits] {n_split} wait insts inserted for:")
        for nm, op, ne in where:
            print(f"  {nm} ({op}): {ne} excess waits")
    return bir


def _build():
    from contextlib import ExitStack

    import concourse.bass as bass
    import concourse.tile as tile
    from concourse import mybir

    f32 = mybir.dt.float32
    bf16 = mybir.dt.bfloat16
    i16 = mybir.dt.int16

    Exp = mybir.ActivationFunctionType.Exp
    Copy = mybir.ActivationFunctionType.Copy

    nc = bass.Bass()
    # all big inputs pre-cast to bf16 host-side: halves DMA traffic and
    # lets the projections run as bf16 matmuls with no on-device casts
    x_kv = nc.dram_tensor("xkv", [C, HW], bf16, kind="ExternalInput")
    wts_d = nc.dram_tensor("wts", [C, 4 * C], bf16, kind="ExternalInput")
    bias_d = nc.dram_tensor("bias", [C, 2], f32, kind="ExternalInput")
    out_d = nc.dram_tensor("out", [C, QH], f32, kind="ExternalOutput")
    den_d = nc.dram_tensor("den", [QH // NBLK, NBLK], f32, kind="ExternalOutput")

    with ExitStack() as ctx:
        tc = ctx.enter_context(tile.TileContext(nc))
        const = ctx.enter_context(tc.tile_pool(name="const", bufs=1))
        persist = ctx.enter_context(tc.tile_pool(name="persist", bufs=1))
        small = ctx.enter_context(tc.tile_pool(name="small", bufs=2))
        pt_pool = ctx.enter_context(tc.tile_pool(name="pt", bufs=16))

        # ---- loads: the DMA bus serializes the whole input set anyway,
        # so use few big transfers on two parallel issue queues ----
        xkv_s = persist.tile([C, HW], bf16, tag="xkv")
        nc.scalar.dma_start(out=xkv_s[:, 0 : HW // 2], in_=x_kv[:, 0 : HW // 2])
        wts_s = const.tile([C, 4 * C], bf16, tag="wts")
        nc.sync.dma_start(out=wts_s, in_=wts_d[:, :])
        bias_s = const.tile([C, 2], f32, tag="bias")
        nc.sync.dma_start(out=bias_s, in_=bias_d[:, :])
        nc.sync.dma_start(out=xkv_s[:, HW // 2 :], in_=x_kv[:, HW // 2 :])
        xkv_t = [xkv_s[:, j * 512 : (j + 1) * 512] for j in range(8)]
        w_s = {
            nm: wts_s[:, i * C : (i + 1) * C]
            for i, nm in enumerate(("wt", "wp", "pg", "co"))
        }
        b_s = {"bt": bias_s[:, 0:1], "bp": bias_s[:, 1:2]}

        # warm the ACT exp table while DMAs stream (one-time ~1.3us load)
        warm = const.tile([C, 1], f32, tag="warm")
        nc.scalar.activation(out=warm, in_=b_s["bt"], func=Exp, bias=0.0, scale=1.0)

        # warm the PE p-state while DMAs stream: ~4us of dummy matmuls on a
        # zeroed scratch tile ramps the clock 0.65 -> 2.4 GHz before the
        # real projections start (output never read; junk by design)
        scratch = const.tile([C, 512], bf16, tag="scratch")
        nc.gpsimd.memset(scratch, 0.0)

        theta_s = persist.tile([C, QH], bf16, tag="theta")
        phi_t = [
            persist.tile([C, QH], bf16, tag=f"phi{t}", name=f"phi{t}")
            for t in range(2)
        ]
        gn_t = [
            persist.tile([C, QH], bf16, tag=f"gn{t}", name=f"gn{t}")
            for t in range(2)
        ]
        gT_t = [
            persist.tile([128, NMCH // 2, 128], bf16, tag=f"gT{t}", name=f"gT{t}")
            for t in range(2)
        ]

        # ---- projections (bf16 512-col matmuls; PSUM->SBUF drains split
        # between ACT and DVE so neither paces the PE stream) ----
        Ident = mybir.ActivationFunctionType.Identity
        with tc.tile_pool(name="proj_ps", bufs=4, space="PSUM") as proj_ps:
            warm_ps = proj_ps.tile([128, 512], f32, tag="warmps")
            for _ in range(16):
                nc.tensor.matmul(warm_ps, scratch[:, 0:128], scratch,
                                 start=True, stop=True)

            def proj(dst, wsrc, j, bias=None, act=None):
                ps = proj_ps.tile([128, 512], f32, tag="p")
                nc.tensor.matmul(
                    ps, w_s[wsrc], xkv_t[j], start=True, stop=True
                )
                if act is None:
                    act = j % 2 == 0
                if act:
                    nc.scalar.activation(
                        out=dst,
                        in_=ps,
                        func=Ident,
                        bias=b_s[bias] if bias else 0.0,
                        scale=1.0,
                    )
                elif bias is not None:
                    nc.vector.tensor_scalar_add(out=dst, in0=ps, scalar1=b_s[bias])
                else:
                    nc.vector.tensor_copy(out=dst, in_=ps)

            for j in range(4):  # theta over this core's queries
                proj(theta_s[:, j * 512 : (j + 1) * 512], "wt", j, "bt")
            for j in range(8):  # phi over all keys
                proj(
                    phi_t[j // 4][:, (j % 4) * 512 : (j % 4 + 1) * 512],
                    "wp",
                    j,
                    "bp",
                )
            for j in range(8):  # ghat natural layout [k, m]
                proj(gn_t[j // 4][:, (j % 4) * 512 : (j % 4 + 1) * 512], "pg", j)
                if j % 4 == 3:
                    # transpose this half: [k=128, 2048] -> [m 128, 16, k 128]
                    half = j // 4
                    nc.sync.dma_start_transpose(out=gT_t[half], in_=gn_t[half])
                    # ones channel -> PV row 0 accumulates the denominator
                    nc.vector.memset(gT_t[half][:, :, 0:1], 1.0)

        # ---- attention ----
        s_pool = ctx.enter_context(tc.tile_pool(name="s_ps", bufs=PVD, space="PSUM"))
        attn_pool = ctx.enter_context(tc.tile_pool(name="attn_ps", bufs=1, space="PSUM"))
        conv_pool = ctx.enter_context(tc.tile_pool(name="conv_ps", bufs=1, space="PSUM"))

        pending = None  # (attn_ps, q0, blk) of the previous block

        def finish_block(attn_ps, q0, blk, last=False):
            den_s = small.tile([1, 512], f32, tag="den_s")
            nc.vector.tensor_copy(out=den_s, in_=attn_ps[0:1, :])
            nc.scalar.dma_start(out=den_d[blk : blk + 1, :], in_=den_s)
            if not last:
                yu = small.tile([128, 512], bf16, tag="yu")
                nc.vector.tensor_copy(out=yu, in_=attn_ps)
                conv_ps = conv_pool.tile([128, 512], f32, tag="conv")
                nc.tensor.matmul(conv_ps, w_s["co"], yu, start=True, stop=True)
                out_s = small.tile([128, 512], f32, tag="out_s")
                nc.vector.tensor_copy(out=out_s, in_=conv_ps)
                nc.sync.dma_start(out=out_d[:, q0 : q0 + NBLK], in_=out_s)
            else:
                # tail: half casts run concurrently on ACT and DVE so the
                # first conv starts ~350ns after the last PV
                conv_ps = conv_pool.tile([128, 512], f32, tag="conv")
                yus = []
                for hh in range(2):
                    sl = slice(hh * 256, (hh + 1) * 256)
                    yu = small.tile([128, 256], bf16, tag=f"yu{hh}", name=f"yu{hh}")
                    if hh == 0:
                        nc.scalar.activation(
                            out=yu, in_=attn_ps[:, sl], func=Copy,
                            bias=0.0, scale=1.0,
                        )
                    else:
                        nc.vector.tensor_copy(out=yu, in_=attn_ps[:, sl])
                    yus.append(yu)
                for hh in range(2):
                    sl = slice(hh * 256, (hh + 1) * 256)
                    nc.tensor.matmul(
                        conv_ps[:, sl], w_s["co"], yus[hh], start=True, stop=True
                    )
                    out_s = small.tile(
                        [128, 256], f32, tag=f"out_s{hh}", name=f"out_s{hh}"
                    )
                    nc.vector.tensor_copy(out=out_s, in_=conv_ps[:, sl])
                    nc.sync.dma_start(
                        out=out_d[:, q0 + hh * 256 : q0 + (hh + 1) * 256],
                        in_=out_s,
                    )

        NPAIR = NMCH // 2
        for blk in range(QH // NBLK):
            q0 = blk * NBLK
            thq = theta_s[:, q0 : q0 + NBLK]
            pt_tiles = []
            attn_ps = attn_pool.tile([128, 512], f32, tag="attn")
            # QK/exp of pair pj runs PVD steps ahead of PV of pair pj-PVD.
            for pj in range(NPAIR + PVD):
                if pj < NPAIR:
                    sp = s_pool.tile([128, 2, 512], f32, tag="s")
                    for k2 in range(2):
                        mi = pj * 2 + k2
                        nc.tensor.matmul(
                            sp[:, k2, :],
                            phi_t[mi // 16][:, (mi % 16) * 128 : (mi % 16 + 1) * 128],
                            thq,
                            start=True,
                            stop=True,
                        )
                    pt = pt_pool.tile([128, 2, 512], bf16, tag="pt")
                    if pj in DVE_PAIRS:
                        # Schraudolph exp on DVE: int16(A*s+B) bits == bf16 P
                        nc.vector.tensor_scalar(
                            out=pt.bitcast(i16),
                            in0=sp,
                            scalar1=EXP_A16,
                            scalar2=EXP_B16,
                            op0=mybir.AluOpType.mult,
                            op1=mybir.AluOpType.add,
                        )
                    else:
                        nc.scalar.activation(
                            out=pt, in_=sp, func=Exp, bias=0.0, scale=1.0
                        )
                    pt_tiles.append(pt)
                if pj == 2 and pending is not None:
                    finish_block(*pending)
                if pj >= PVD:
                    p = pj - PVD
                    for k2 in range(2):
                        mi = p * 2 + k2
                        nc.tensor.matmul(
                            attn_ps,
                            gT_t[mi // 16][:, mi % 16, :],
                            pt_tiles[p][:, k2, :],
                            start=(mi == 0),
                            stop=(mi == NMCH - 1),
                        )
            pending = (attn_ps, q0, blk)
        finish_block(*pending, last=True)

    # populate .instr bytes for extended-inst InstISA subclasses — raw Bass
    # skips this pass and the NEFF compiler fails "ISA wrong length"
    mybir.codegen_inst_isa_subclasses(nc)

    import json as _json
    import os as _os

    blob = _json.dumps(
        _legalize_waits(
            _json.loads(nc.to_json_bytes()),
            verbose=bool(_os.environ.get("KERNEL_DEBUG")),
        )
    ).encode()
    nc.to_json_bytes = lambda: blob
    return nc


def _get_nc():
    if "nc" not in _CACHE:
        _CACHE["nc"] = _build()
    return _CACHE["nc"]


def _prep_host(inputs):
    """Host-side precompute: weight transposes, fused G = w_out@w_g SVD
    split (rank 127 + ones/denominator channel at k=0), fused bias, and
    bf16 casts + packing of all device weight inputs."""
    import ml_dtypes

    bf16 = ml_dtypes.bfloat16
    w_g = np.asarray(inputs["w_g"], np.float32)
    w_out = np.asarray(inputs["w_out"], np.float32)
    G = w_out @ w_g
    U, S, Vt = np.linalg.svd(G)
    r = 127
    pg = np.zeros((C, C), np.float32)  # lhsT: pg[c, k] = P_g[k-1, c]
    pg[:, 1 : r + 1] = Vt[:r, :].T
    co = np.zeros((C, C), np.float32)  # lhsT: co[k, c] = C_out[c, k-1]
    co[1 : r + 1, :] = (U[:, :r] * S[:r][None, :]).T
    bcomb = (
        np.asarray(inputs["b_out"], np.float32)
        + w_out @ np.asarray(inputs["b_g"], np.float32)
    ).reshape(C, 1)
    wts = np.concatenate(
        [
            np.asarray(inputs["w_theta"], np.float32).T,
            np.asarray(inputs["w_phi"], np.float32).T,
            pg,
            co,
        ],
        axis=1,
    ).astype(bf16)
    bias = np.concatenate(
        [
            np.asarray(inputs["b_theta"], np.float32).reshape(C, 1),
            np.asarray(inputs["b_phi"], np.float32).reshape(C, 1),
        ],
        axis=1,
    )
    wmaps = {
        "wts": np.ascontiguousarray(wts),
        "bias": np.ascontiguousarray(bias),
    }
    return wmaps, bcomb


def _run(inputs, trace=False, **spmd_kwargs):
    import ml_dtypes

    from concourse.bass_utils import run_bass_kernel_spmd

    x = np.asarray(inputs["x"], np.float32)
    xf = np.ascontiguousarray(x.reshape(B, C, HW))
    wmaps, bcomb = _prep_host(inputs)
    in_maps = []
    for k in range(NCORES):
        b, h = k // 2, k % 2
        # rotate keys so this core's queries are columns [0, QH)
        xkv = np.ascontiguousarray(
            np.roll(xf[b], -h * QH, axis=1).astype(ml_dtypes.bfloat16)
        )
        in_maps.append({"xkv": xkv, **wmaps})
    nc = _get_nc()
    res = run_bass_kernel_spmd(
        nc, in_maps, core_ids=list(range(NCORES)), trace=trace, **spmd_kwargs
    )
    out = np.empty((B, C, HW), np.float32)
    for k in range(NCORES):
        b, h = k // 2, k % 2
        conv_u = res.results[k]["out"]  # [C, QH], unnormalized conv result
        den = res.results[k]["den"].reshape(QH)  # softmax denominators
        xq = xf[b][:, h * QH : (h + 1) * QH]
        out[b][:, h * QH : (h + 1) * QH] = conv_u / den[None, :] + xq + bcomb
    return out.reshape(B, C, 64, 64), res


def kernel(**inputs):
    out, _ = _run(inputs, trace=False)
    return out


# revision 44
# speedup vs baseline: 1.0632x; 1.0066x over previous
"""NonLocalBlock (spatial self-attention) Trainium2 Bass kernel.

Problem: x [4, 128, 64, 64]; 1x1 convs theta/phi/g -> softmax(theta^T phi) g
-> 1x1 conv out + residual.

Sharding (8 cores): core k -> (batch b = k//2, query-half h = k%2).
Each core holds the full keys/values for its batch (xkv [128, 4096], rolled
host-side so its 2048 queries are columns [0, 2048)).  Weights replicated.

Key structural ideas:

1. Fused value path, rank-127:  G = w_out @ w_g has sigma_128 ~ 1e-9, so
   G ~= C_out @ P_g with P_g = V^T[:127] and C_out = U[:, :127] * S[:127].
   The PV stationary chunks [m=128, 128] hold column 0 = ones and columns
   1..127 = (P_g x)^T, so a single PV matmul accumulates BOTH the attention
   value sum (rows 1..127) and the softmax denominator (row 0).  No
   dedicated denominator matmuls or reductions anywhere.

2. Host-side normalization:  out = C_out(y/den) + x + b == (C_out y)/den
   + x + b, so the device ships the *unnormalized* conv result and the den
   row; the host does conv/den + x + b in numpy.  No reciprocal /
   partition-broadcast round-trip on device.

3. Two-engine exp: ACT computes exp for 10 of every 16 key-chunk pairs;
   DVE computes the other 6 with a Schraudolph bit-trick in ONE
   tensor_scalar op: i16 = round(s * 128*log2(e) + (127*128 - C)), whose
   int16 bit pattern IS bf16(exp(s)) (~3% max element error, common-mode
   across neighbouring scores so softmax normalization cancels most of it;
   end-to-end sim: 5-6e-3 rel err).  This removes ACT as the pipeline
   pacer; the PE's 512-column matmul stream is the bottleneck.

4. p-state care: TRN2's PE ramps 0.65/1.2 -> 2.4 GHz only after ~3us of
   gapless execution.  Dummy matmuls on a zeroed scratch tile ramp the
   clock while the input DMAs stream, the bf16 projections (host-precast
   x and weights, so no on-device cast chain) keep it hot, and QK runs 3
   pair-steps ahead of exp (s_pool bufs=3, PV delayed 3) so ACT/DVE
   jitter never starves the PE.

Per 512-query block (16 key-chunk pairs, software-pipelined):
  S^T pair [128m, 2, 512n] = phi_chunk^T @ theta_blk  (PSUM, bf16)
  P^T = exp(S^T)  (ACT or DVE, PSUM->SBUF bf16; scores O(30) safe in fp32)
  attn_ps [128, 512] += ghatT_chunk^T @ P^T_chunk  (PSUM accum, bf16)
  epilogue of block b (bf16 cast, conv, DMA out) emitted early in block b+1.
"""

import numpy as np

B, C = 4, 128
HW = 4096  # 64*64 spatial positions
QH = HW // 2  # queries per core
NCORES = 8
NBLK = 512  # query block size
NMCH = HW // 128  # 32 key chunks of 128
PVD = 3  # PV trails QK by this many pair-steps (= s_pool bufs)
DVE_PAIRS = {2, 4, 7, 9, 12, 14}  # pair indices handled by the DVE exp

# Schraudolph constants for bf16-via-int16: bitcast_bf16(round_i16(A*s + B))
EXP_A16 = 184.6649652337873  # 2^7 * log2(e)
EXP_B16 = 16250.409332        # 127*128 - 366392.7/65536

_CACHE = {}


def _legalize_waits(bir, verbose=False):
    """Split instructions carrying more sync waits than the gen3 ISA allows.

    Walrus caps sync waits at 1 per instruction (2 for EventSemaphore); the
    Tile tail drain and first-consumer instructions can exceed that. Spill
    excess waits onto inserted wait-only EventSemaphore instructions placed
    immediately before the offender on the same engine (engines execute
    in order, so this is semantics-preserving).
    """
    n_split = 0
    where = []
    for f in bir["functions"]:
        for bb in f["blocks"]:
            out = []
            for inst in bb["instructions"]:
                si = inst.get("sync_info")
                waits = (si or {}).get("on_wait") or []
                cap = 2 if inst["opcode"] == "EventSemaphore" else 1
                if len(waits) > cap:
                    excess = waits[:-cap]
                    si["on_wait"] = waits[-cap:]
                    for i in range(0, len(excess), 2):
                        chunk = excess[i : i + 2]
                        out.append(
                            {
                                "debug": inst.get("debug", 0),
                                "engine": inst["engine"],
                                "ins": [],
                                "name": f'{inst["name"]}_w{i}',
                                "opcode": "EventSemaphore",
                                "outs": [],
                                "sync_info": {"on_update": [], "on_wait": chunk},
                            }
                        )
                        n_split += 1
                    where.append((inst["name"], inst["opcode"], len(excess)))
                out.append(inst)
            bb["instructions"] = out
    if verbose and where:
        print(f"[legalize_waits] {n_split} wait insts inserted for:")
        for nm, op, ne in where:
            print(f"  {nm} ({op}): {ne} excess waits")
    return bir


def _build():
    from contextlib import ExitStack

    import concourse.bass as bass
    import concourse.tile as tile
    from concourse import mybir

    f32 = mybir.dt.float32
    bf16 = mybir.dt.bfloat16
    i16 = mybir.dt.int16

    Exp = mybir.ActivationFunctionType.Exp
    Copy = mybir.ActivationFunctionType.Copy

    nc = bass.Bass()
    # all big inputs pre-cast to bf16 host-side: halves DMA traffic and
    # lets the projections run as bf16 matmuls with no on-device casts
    x_kv = nc.dram_tensor("xkv", [C, HW], bf16, kind="ExternalInput")
    wts_d = nc.dram_tensor("wts", [C, 4 * C], bf16, kind="ExternalInput")
    bias_d = nc.dram_tensor("bias", [C, 2], f32, kind="ExternalInput")
    out_d = nc.dram_tensor("out", [C, QH], f32, kind="ExternalOutput")
    den_d = nc.dram_tensor("den", [QH // NBLK, NBLK], f32, kind="ExternalOutput")

    with ExitStack() as ctx:
        tc = ctx.enter_context(tile.TileContext(nc))
        const = ctx.enter_context(tc.tile_pool(name="const", bufs=1))
        persist = ctx.enter_context(tc.tile_pool(name="persist", bufs=1))
        small = ctx.enter_context(tc.tile_pool(name="small", bufs=2))
        pt_pool = ctx.enter_context(tc.tile_pool(name="pt", bufs=16))

        # ---- loads: the DMA bus serializes the whole input set anyway,
        # so use few big transfers on two parallel issue queues ----
        xkv_s = persist.tile([C, HW], bf16, tag="xkv")
        nc.scalar.dma_start(out=xkv_s[:, 0 : HW // 2], in_=x_kv[:, 0 : HW // 2])
        wts_s = const.tile([C, 4 * C], bf16, tag="wts")
        nc.sync.dma_start(out=wts_s, in_=wts_d[:, :])
        bias_s = const.tile([C, 2], f32, tag="bias")
        nc.sync.dma_start(out=bias_s, in_=bias_d[:, :])
        nc.sync.dma_start(out=xkv_s[:, HW // 2 :], in_=x_kv[:, HW // 2 :])
        xkv_t = [xkv_s[:, j * 512 : (j + 1) * 512] for j in range(8)]
        w_s = {
            nm: wts_s[:, i * C : (i + 1) * C]
            for i, nm in enumerate(("wt", "wp", "pg", "co"))
        }
        b_s = {"bt": bias_s[:, 0:1], "bp": bias_s[:, 1:2]}

        # warm the ACT exp table while DMAs stream (one-time ~1.3us load)
        warm = const.tile([C, 1], f32, tag="warm")
        nc.scalar.activation(out=warm, in_=b_s["bt"], func=Exp, bias=0.0, scale=1.0)

        # warm the PE p-state while DMAs stream: ~4us of dummy matmuls on a
        # zeroed scratch tile ramps the clock 0.65 -> 2.4 GHz before the
        # real projections start (output never read; junk by design)
        scratch = const.tile([C, 512], bf16, tag="scratch")
        nc.gpsimd.memset(scratch, 0.0)

        theta_s = persist.tile([C, QH], bf16, tag="theta")
        phi_t = [
            persist.tile([C, QH], bf16, tag=f"phi{t}", name=f"phi{t}")
            for t in range(2)
        ]
        gn_t = [
            persist.tile([C, QH], bf16, tag=f"gn{t}", name=f"gn{t}")
            for t in range(2)
        ]
        gT_t = [
            persist.tile([128, NMCH // 2, 128], bf16, tag=f"gT{t}", name=f"gT{t}")
            for t in range(2)
        ]

        # ---- projections (bf16 512-col matmuls; PSUM->SBUF drains split
        # between ACT and DVE so neither paces the PE stream) ----
        Ident = mybir.ActivationFunctionType.Identity
        with tc.tile_pool(name="proj_ps", bufs=4, space="PSUM") as proj_ps:
            warm_ps = proj_ps.tile([128, 512], f32, tag="warmps")
            for _ in range(12):
                nc.tensor.matmul(warm_ps, scratch[:, 0:128], scratch,
                                 start=True, stop=True)

            def proj(dst, wsrc, j, bias=None, act=None):
                ps = proj_ps.tile([128, 512], f32, tag="p")
                nc.tensor.matmul(
                    ps, w_s[wsrc], xkv_t[j], start=True, stop=True
                )
                if act is None:
                    act = j % 2 == 0
                if act:
                    nc.scalar.activation(
                        out=dst,
                        in_=ps,
                        func=Ident,
                        bias=b_s[bias] if bias else 0.0,
                        scale=1.0,
                    )
                elif bias is not None:
                    nc.vector.tensor_scalar_add(out=dst, in0=ps, scalar1=b_s[bias])
                else:
                    nc.vector.tensor_copy(out=dst, in_=ps)

            for j in range(4):  # theta over this core's queries
                proj(theta_s[:, j * 512 : (j + 1) * 512], "wt", j, "bt")
            for j in range(8):  # phi over all keys
                proj(
                    phi_t[j // 4][:, (j % 4) * 512 : (j % 4 + 1) * 512],
                    "wp",
                    j,
                    "bp",
                )
            for j in range(8):  # ghat natural layout [k, m]
                proj(gn_t[j // 4][:, (j % 4) * 512 : (j % 4 + 1) * 512], "pg", j)
                if j % 4 == 3:
                    # transpose this half: [k=128, 2048] -> [m 128, 16, k 128]
                    half = j // 4
                    nc.sync.dma_start_transpose(out=gT_t[half], in_=gn_t[half])
                    # ones channel -> PV row 0 accumulates the denominator
                    nc.vector.memset(gT_t[half][:, :, 0:1], 1.0)

        # ---- attention ----
        s_pool = ctx.enter_context(tc.tile_pool(name="s_ps", bufs=PVD, space="PSUM"))
        attn_pool = ctx.enter_context(tc.tile_pool(name="attn_ps", bufs=1, space="PSUM"))
        conv_pool = ctx.enter_context(tc.tile_pool(name="conv_ps", bufs=1, space="PSUM"))

        pending = None  # (attn_ps, q0, blk) of the previous block

        def finish_block(attn_ps, q0, blk, last=False):
            den_s = small.tile([1, 512], f32, tag="den_s")
            nc.vector.tensor_copy(out=den_s, in_=attn_ps[0:1, :])
            nc.scalar.dma_start(out=den_d[blk : blk + 1, :], in_=den_s)
            if not last:
                yu = small.tile([128, 512], bf16, tag="yu")
                nc.vector.tensor_copy(out=yu, in_=attn_ps)
                conv_ps = conv_pool.tile([128, 512], f32, tag="conv")
                nc.tensor.matmul(conv_ps, w_s["co"], yu, start=True, stop=True)
                out_s = small.tile([128, 512], f32, tag="out_s")
                nc.vector.tensor_copy(out=out_s, in_=conv_ps)
                nc.sync.dma_start(out=out_d[:, q0 : q0 + NBLK], in_=out_s)
            else:
                # tail: half casts run concurrently on ACT and DVE so the
                # first conv starts ~350ns after the last PV
                conv_ps = conv_pool.tile([128, 512], f32, tag="conv")
                yus = []
                for hh in range(2):
                    sl = slice(hh * 256, (hh + 1) * 256)
                    yu = small.tile([128, 256], bf16, tag=f"yu{hh}", name=f"yu{hh}")
                    if hh == 0:
                        nc.scalar.activation(
                            out=yu, in_=attn_ps[:, sl], func=Copy,
                            bias=0.0, scale=1.0,
                        )
                    else:
                        nc.vector.tensor_copy(out=yu, in_=attn_ps[:, sl])
                    yus.append(yu)
                for hh in range(2):
                    sl = slice(hh * 256, (hh + 1) * 256)
                    nc.tensor.matmul(
                        conv_ps[:, sl], w_s["co"], yus[hh], start=True, stop=True
                    )
                    out_s = small.tile(
                        [128, 256], f32, tag=f"out_s{hh}", name=f"out_s{hh}"
                    )
                    nc.vector.tensor_copy(out=out_s, in_=conv_ps[:, sl])
                    nc.sync.dma_start(
                        out=out_d[:, q0 + hh * 256 : q0 + (hh + 1) * 256],
                        in_=out_s,
                    )

        NPAIR = NMCH // 2
        for blk in range(QH // NBLK):
            q0 = blk * NBLK
            thq = theta_s[:, q0 : q0 + NBLK]
            pt_tiles = []
            attn_ps = attn_pool.tile([128, 512], f32, tag="attn")
            # QK/exp of pair pj runs PVD steps ahead of PV of pair pj-PVD.
            for pj in range(NPAIR + PVD):
                if pj < NPAIR:
                    sp = s_pool.tile([128, 2, 512], f32, tag="s")
                    for k2 in range(2):
                        mi = pj * 2 + k2
                        nc.tensor.matmul(
                            sp[:, k2, :],
                            phi_t[mi // 16][:, (mi % 16) * 128 : (mi % 16 + 1) * 128],
                            thq,
                            start=True,
                            stop=True,
                        )
                    pt = pt_pool.tile([128, 2, 512], bf16, tag="pt")
                    if pj in DVE_PAIRS:
                        # Schraudolph exp on DVE: int16(A*s+B) bits == bf16 P
                        nc.vector.tensor_scalar(
                            out=pt.bitcast(i16),
                            in0=sp,
                            scalar1=EXP_A16,
                            scalar2=EXP_B16,
                            op0=mybir.AluOpType.mult,
                            op1=mybir.AluOpType.add,
                        )
                    else:
                        nc.scalar.activation(
                            out=pt, in_=sp, func=Exp, bias=0.0, scale=1.0
                        )
                    pt_tiles.append(pt)
                if pj == 1 and pending is not None:
                    finish_block(*pending)
                if pj >= PVD:
                    p = pj - PVD
                    for k2 in range(2):
                        mi = p * 2 + k2
                        nc.tensor.matmul(
                            attn_ps,
                            gT_t[mi // 16][:, mi % 16, :],
                            pt_tiles[p][:, k2, :],
                            start=(mi == 0),
                            stop=(mi == NMCH - 1),
                        )
            pending = (attn_ps, q0, blk)
        finish_block(*pending, last=True)

    # populate .instr bytes for extended-inst InstISA subclasses — raw Bass
    # skips this pass and the NEFF compiler fails "ISA wrong length"
    mybir.codegen_inst_isa_subclasses(nc)

    import json as _json
    import os as _os

    blob = _json.dumps(
        _legalize_waits(
            _json.loads(nc.to_json_bytes()),
            verbose=bool(_os.environ.get("KERNEL_DEBUG")),
        )
    ).encode()
    nc.to_json_bytes = lambda: blob
    return nc


def _get_nc():
    if "nc" not in _CACHE:
        _CACHE["nc"] = _build()
    return _CACHE["nc"]


def _prep_host(inputs):
    """Host-side precompute: weight transposes, fused G = w_out@w_g SVD
    split (rank 127 + ones/denominator channel at k=0), fused bias, and
    bf16 casts + packing of all device weight inputs."""
    import ml_dtypes

    bf16 = ml_dtypes.bfloat16
    w_g = np.asarray(inputs["w_g"], np.float32)
    w_out = np.asarray(inputs["w_out"], np.float32)
    G = w_out @ w_g
    U, S, Vt = np.linalg.svd(G)
    r = 127
    pg = np.zeros((C, C), np.float32)  # lhsT: pg[c, k] = P_g[k-1, c]
    pg[:, 1 : r + 1] = Vt[:r, :].T
    co = np.zeros((C, C), np.float32)  # lhsT: co[k, c] = C_out[c, k-1]
    co[1 : r + 1, :] = (U[:, :r] * S[:r][None, :]).T
    bcomb = (
        np.asarray(inputs["b_out"], np.float32)
        + w_out @ np.asarray(inputs["b_g"], np.float32)
    ).reshape(C, 1)
    wts = np.concatenate(
        [
            np.asarray(inputs["w_theta"], np.float32).T,
            np.asarray(inputs["w_phi"], np.float32).T,
            pg,
            co,
        ],
        axis=1,
    ).astype(bf16)
    bias = np.concatenate(
        [
            np.asarray(inputs["b_theta"], np.float32).reshape(C, 1),
            np.asarray(inputs["b_phi"], np.float32).reshape(C, 1),
        ],
        axis=1,
    )
    wmaps = {
        "wts": np.ascontiguousarray(wts),
        "bias": np.ascontiguousarray(bias),
    }
    return wmaps, bcomb


def _run(inputs, trace=False, **spmd_kwargs):
    import ml_dtypes

    from concourse.bass_utils import run_bass_kernel_spmd

    x = np.asarray(inputs["x"], np.float32)
    xf = np.ascontiguousarray(x.reshape(B, C, HW))
    wmaps, bcomb = _prep_host(inputs)
    in_maps = []
    for k in range(NCORES):
        b, h = k // 2, k % 2
        # rotate keys so this core's queries are columns [0, QH)
        xkv = np.ascontiguousarray(
            np.roll(xf[b], -h * QH, axis=1).astype(ml_dtypes.bfloat16)
        )
        in_maps.append({"xkv": xkv, **wmaps})
    nc = _get_nc()
    res = run_bass_kernel_spmd(
        nc, in_maps, core_ids=list(range(NCORES)), trace=trace, **spmd_kwargs
    )
    out = np.empty((B, C, HW), np.float32)
    for k in range(NCORES):
        b, h = k // 2, k % 2
        conv_u = res.results[k]["out"]  # [C, QH], unnormalized conv result
        den = res.results[k]["den"].reshape(QH)  # softmax denominators
        xq = xf[b][:, h * QH : (h + 1) * QH]
        out[b][:, h * QH : (h + 1) * QH] = conv_u / den[None, :] + xq + bcomb
    return out.reshape(B, C, 64, 64), res


def kernel(**inputs):
    out, _ = _run(inputs, trace=False)
    return out
